# revision 20
# baseline (speedup 1.0000x reference)
"""Causal self-attention (B=2, T=2048, C=2048, H=16) on 8 TRN2 NeuronCores.

Sharding: tensor-parallel over heads (2 heads per core, both batches on every
core). Each core computes q/k/v projections for its 2 heads, RoPE, causal
softmax(qk^T)v, and a partial output projection against its slice of Wo's
columns. The host sums the 8 partial projections and adds the (linear) bias
terms.

v2 layout strategy (evidence: PE streams at 1 cyc/row when fed; all loss was
inter-instruction stalls — attention locally ACT-bound, out-proj drain-bound,
slow gpsimd casting DMAs starving prefetch):
  - x & qkv weights stay fp32 (exact projections); tiles are declared f32 and
    bitcast to f32r at the matmul (1 cyc/row for N>=256), no casting DMAs.
  - everything the attention path touches is bf16: q/k post-RoPE, exp(P),
    v, y, Wo. bf16 matmuls are 1 cyc/row at any N and halve LDWEIGHTS;
    bf16 DVE ops run in 2x mode.
  - emission order gives the Tile dataflow scheduler cross-phase overlap:
    attention(i) + out-proj(i) are emitted before qkv-proj(i+1), so PE fills
    exp-latency gaps with next-block projection chains.
  - PSUM budget exactly 8 banks: ms=2 (scores/den/out-proj), my=2 (y acc),
    prj=2 (qk proj), vps=2 (v proj).
  - engine split: ACT = exp + q/k bias moves + half the PSUM drains,
    DVE = RoPE (bf16 2x) + dacc (f32 += bf16) + mask + recip + yT + rest.
  - denominator: dacc accumulated on DVE, then one ones-matmul broadcasts
    column sums to all partitions (exact fp32).
  - output written bf16 (halves store traffic); host sums 8 partials in f32.
"""

import sys

sys.path.insert(0, "/opt/trn_rl_repo")

import numpy as np
import ml_dtypes

import concourse.bacc as bacc
import concourse.mybir as mybir
import concourse.tile as tile
from concourse import bass_utils

B, T, C, H = 2, 2048, 2048, 16
HD = C // H  # 128
BASE = 10000.0
NC_ = 8  # cores
NH = H // NC_  # heads per core = 2
TB = 512  # T block
NTB = T // TB  # 4
CK = C // 128  # 16 contraction chunks
SCALE = 1.0 / float(np.sqrt(np.float32(HD)))

f32 = mybir.dt.float32
f32r = mybir.dt.float32r
bf16 = mybir.dt.bfloat16
AF = mybir.ActivationFunctionType
OP = mybir.AluOpType
BF16 = ml_dtypes.bfloat16

TRACE = False
LAST_RESULT = None

_STATE = {}


def _rope_tables():
    """cos/sin tables [HD, T] mirroring reference._rope_tables (f32 chain)."""
    try:
        import jax
        import jax.numpy as jnp

        cpu = jax.devices("cpu")[0]
        with jax.default_device(cpu):
            p = jnp.arange(HD // 2, dtype=jnp.float32)
            theta = jnp.power(BASE, -(2.0**p) / HD)
            pos = jnp.arange(1, T + 1, dtype=jnp.float32)[:, None]
            c = pos * theta
            ang = jnp.concatenate([c, c], axis=-1)  # [T, HD]
            cos = np.asarray(jnp.cos(ang)).T  # [HD, T]
            sin = np.asarray(jnp.sin(ang)).T
        return np.ascontiguousarray(cos), np.ascontiguousarray(sin)
    except Exception:
        p = np.arange(HD // 2, dtype=np.float32)
        theta = np.power(np.float32(BASE), (-(2.0**p) / HD).astype(np.float32))
        pos = np.arange(1, T + 1, dtype=np.float32)[:, None]
        c = (pos * theta).astype(np.float32)
        ang = np.concatenate([c, c], axis=-1)
        return (
            np.ascontiguousarray(np.cos(ang).T.astype(np.float32)),
            np.ascontiguousarray(np.sin(ang).T.astype(np.float32)),
        )


def _build_program():
    nc = bacc.Bacc("TRN2", target_bir_lowering=False, debug=False, num_devices=NC_)

    d_xT = nc.dram_tensor("xT", (B, 128, CK, T), bf16, kind="ExternalInput")
    d_wq = nc.dram_tensor("wq", (128, CK, NH * HD), bf16, kind="ExternalInput")
    d_wk = nc.dram_tensor("wk", (128, CK, NH * HD), bf16, kind="ExternalInput")
    d_wv = nc.dram_tensor("wv", (128, CK, NH * HD), bf16, kind="ExternalInput")
    d_wo = nc.dram_tensor("wo", (NH * HD, C), bf16, kind="ExternalInput")
    d_bq = nc.dram_tensor("bq", (HD, NH), f32, kind="ExternalInput")
    d_bk = nc.dram_tensor("bk", (HD, NH), f32, kind="ExternalInput")
    d_cos = nc.dram_tensor("cosT", (HD, T), bf16, kind="ExternalInput")
    d_sin = nc.dram_tensor("sinT", (HD, T), bf16, kind="ExternalInput")
    d_mask = nc.dram_tensor("mask0", (128, 128), bf16, kind="ExternalInput")
    d_ones = nc.dram_tensor("onesm", (128, 128), f32r, kind="ExternalInput")
    d_out = nc.dram_tensor("out", (B, T, C), bf16, kind="ExternalOutput")

    blocks = [(b, tb) for b in range(B) for tb in range(NTB)]

    with tile.TileContext(nc) as tc:
        with (
            tc.tile_pool(name="w", bufs=1) as wp,
            tc.tile_pool(name="xp", bufs=1) as xp,
            tc.tile_pool(name="kv", bufs=1) as kvp,
            tc.tile_pool(name="work", bufs=1) as wk_,
            tc.tile_pool(name="ps", bufs=1, space="PSUM") as ps,
        ):
            # --- resident weights/constants (plain sync DMAs) ---
            wq_t = wp.tile([128, CK, NH * HD], bf16, name="wq_t")
            wk_t = wp.tile([128, CK, NH * HD], bf16, name="wk_t")
            wv_t = wp.tile([128, CK, NH * HD], bf16, name="wv_t")
            bq_t = wp.tile([128, NH], f32, name="bq_t")
            nc.sync.dma_start(bq_t[:], d_bq.ap()[:])
            bk_t = wp.tile([128, NH], f32, name="bk_t")
            nc.sync.dma_start(bk_t[:], d_bk.ap()[:])
            cos_t = wp.tile([128, T], bf16, name="cos_t")
            sin_t = wp.tile([128, T], bf16, name="sin_t")
            nc.sync.dma_start(cos_t[:], d_cos.ap()[:])
            nc.sync.dma_start(sin_t[:], d_sin.ap()[:])
            mask_t = wp.tile([128, 128], bf16, name="mask_t")
            nc.sync.dma_start(mask_t[:], d_mask.ap()[:])
            ones_t = wp.tile([128, 128], f32r, name="ones_t")
            nc.sync.dma_start(ones_t[:], d_ones.ap()[:])

            state = {}

            def emit_x(i):
                b, tb = blocks[i]
                tbs = slice(tb * TB, (tb + 1) * TB)
                xt = xp.tile([128, CK, TB], bf16, tag="xtb", bufs=2, name=f"xt_{i}")
                nc.sync.dma_start(xt[:], d_xT.ap()[b, :, :, tbs])
                return [xt[:, kc, :] for kc in range(CK)]

            def emit_qkv_proj(i, xts, after_qk=None):
                b, tb = blocks[i]
                tbs = slice(tb * TB, (tb + 1) * TB)
                if tb == 0:
                    state[("kts", b)] = [
                        kvp.tile(
                            [128, T], bf16, tag=f"kt{h}", bufs=2, name=f"kt{h}_{b}"
                        )
                        for h in range(NH)
                    ]
                    state[("vt", b)] = kvp.tile(
                        [128, T // 128, NH * HD], bf16, tag="v", bufs=2, name=f"v_{b}"
                    )
                kts = state[("kts", b)]
                vt = state[("vt", b)]
                qTs = []
                for h in range(NH):
                    hsl = slice(h * HD, (h + 1) * HD)
                    qT = wk_.tile([128, TB], bf16, tag="q", bufs=4)
                    for (w_t, b_t, dest) in (
                        (wq_t, bq_t, qT[:]),
                        (wk_t, bk_t, kts[h][:, tbs]),
                    ):
                        prj = ps.tile([128, TB], f32, tag="prj", bufs=2)
                        for kc in range(CK):
                            nc.tensor.matmul(
                                prj[:],
                                w_t[:, kc, hsl],
                                xts[kc],
                                start=(kc == 0),
                                stop=(kc == CK - 1),
                            )
                        qb = wk_.tile([128, TB], bf16, tag="qb", bufs=3)
                        nc.scalar.activation(
                            qb[:], prj[:], AF.Identity, bias=b_t[:, h : h + 1]
                        )
                        t1 = wk_.tile([128, TB], bf16, tag="rtmp", bufs=4)
                        nc.vector.tensor_tensor(t1[:], qb[:], cos_t[:, tbs], OP.mult)
                        t2 = wk_.tile([128, TB], bf16, tag="rtmp", bufs=4)
                        nc.vector.tensor_tensor(
                            t2[0:64, :], qb[64:128, :], sin_t[64:128, tbs], OP.mult
                        )
                        nc.vector.tensor_tensor(
                            t2[64:128, :], qb[0:64, :], sin_t[0:64, tbs], OP.mult
                        )
                        nc.vector.tensor_tensor(dest, t1[:], t2[:], OP.add)
                    qTs.append(qT)
                if after_qk is not None:
                    after_qk()
                # ---- v projection (both heads together, N=256) ----
                for tt in range(4):
                    vps = ps.tile([128, NH * HD], f32, tag="vps", bufs=2)
                    for kc in range(CK):
                        nc.tensor.matmul(
                            vps[:],
                            xts[kc][:, tt * 128 : (tt + 1) * 128],
                            wv_t[:, kc, :],
                            start=(kc == 0),
                            stop=(kc == CK - 1),
                        )
                    if tt % 2 == 0:
                        nc.scalar.activation(vt[:, tb * 4 + tt, :], vps[:], AF.Identity)
                    else:
                        nc.vector.tensor_copy(vt[:, tb * 4 + tt, :], vps[:])
                state[("qTs", i)] = qTs

            def emit_attention(i):
                b, tb = blocks[i]
                kts = state[("kts", b)]
                vt = state[("vt", b)]
                qTs = state.pop(("qTs", i))
                nkt = 4 * tb + 4
                # diagonal (W=512, masked) tile first so accumulation chains
                # start full-width; remaining diagonals at the end
                kt_order = (
                    [4 * tb] + list(range(4 * tb)) + [4 * tb + 1, 4 * tb + 2, 4 * tb + 3]
                )
                y_pss = [
                    ps.tile([128, TB], f32, tag="my", bufs=2, name=f"yps{h}_{i}")
                    for h in range(NH)
                ]
                daccs = [
                    wk_.tile([128, TB], f32r, tag="dacc", bufs=3, name=f"dacc{h}_{i}")
                    for h in range(NH)
                ]
                for idx, kt in enumerate(kt_order):
                    o = kt - 4 * tb
                    q0 = 128 * o if o > 0 else 0
                    W = TB - q0
                    first = idx == 0
                    last = idx == nkt - 1
                    for h in range(NH):
                        hsl = slice(h * HD, (h + 1) * HD)
                        s_ps = ps.tile([128, TB], f32, tag="ms", bufs=2)
                        nc.tensor.matmul(
                            s_ps[:, :W],
                            kts[h][:, kt * 128 : (kt + 1) * 128],
                            qTs[h][:, q0:],
                        )
                        pt = wk_.tile([128, TB], bf16, tag="p", bufs=6)
                        nc.scalar.activation(pt[:, :W], s_ps[:, :W], AF.Exp, scale=SCALE)
                        if o >= 0:
                            # triangular chunk is the first 128 live columns
                            nc.vector.tensor_tensor(
                                pt[:, :128], pt[:, :128], mask_t[:], OP.mult
                            )
                        nc.tensor.matmul(
                            y_pss[h][:, q0:],
                            vt[:, kt, hsl],
                            pt[:, :W],
                            start=first,
                            stop=last,
                            skip_group_check=True,
                        )
                        if first:
                            nc.vector.tensor_copy(daccs[h][:], pt[:])
                        else:
                            nc.vector.tensor_tensor(
                                daccs[h][:, q0:],
                                daccs[h][:, q0:].bitcast(f32),
                                pt[:, :W],
                                OP.add,
                            )
                yTs = []
                for h in range(NH):
                    den_ps = ps.tile([128, TB], f32, tag="ms", bufs=2)
                    nc.tensor.matmul(
                        den_ps[:], ones_t[:], daccs[h][:]
                    )
                    rden = wk_.tile([128, TB], f32, tag="rden", bufs=2)
                    nc.vector.reciprocal_approx_fast(rden[:], den_ps[:])
                    yT = wk_.tile([128, TB], bf16, tag="y", bufs=5)
                    nc.vector.tensor_tensor(yT[:], y_pss[h][:], rden[:], OP.mult)
                    yTs.append(yT)
                return yTs

            def emit_outproj(i, yTs):
                b, tb = blocks[i]
                last_block = i == len(blocks) - 1
                for tt in range(4):
                    r0 = tb * TB + tt * 128
                    ot = wk_.tile([128, C], bf16, tag="o", bufs=3)
                    for ncc in range(4):
                        o_ps = ps.tile([128, TB], f32, tag="ms", bufs=2)
                        for h in range(NH):
                            nc.tensor.matmul(
                                o_ps[:],
                                yTs[h][:, tt * 128 : (tt + 1) * 128],
                                wo_t[:, h, ncc * TB : (ncc + 1) * TB],
                                start=(h == 0),
                                stop=(h == NH - 1),
                            )
                        osl = ot[:, ncc * TB : (ncc + 1) * TB]
                        if ncc % 2 == 0:
                            nc.scalar.activation(osl, o_ps[:], AF.Identity)
                        else:
                            nc.vector.tensor_copy(osl, o_ps[:])
                        if last_block:
                            # drain per chunk so the final stores overlap the
                            # remaining copies instead of serializing after them
                            nc.sync.dma_start(
                                d_out.ap()[
                                    b, r0 : r0 + 128, ncc * TB : (ncc + 1) * TB
                                ],
                                osl,
                            )
                    if not last_block:
                        nc.sync.dma_start(d_out.ap()[b, r0 : r0 + 128, :], ot[:])

            # prologue: x(0) + wq/wk interleaved per chunk (fast time-to-first-
            # matmul), deferred wv/wo loads
            xts0 = []
            xg = []
            for kc0 in range(0, CK, 4):
                xt = xp.tile([128, 4, TB], bf16, tag="xt", bufs=8, name=f"xt0_{kc0}")
                xg.append(xt)
                xts0.extend(xt[:, g, :] for g in range(4))
            nc.sync.dma_start(wq_t[:, 0:2, :], d_wq.ap()[:, 0:2, :])
            nc.sync.dma_start(xg[0][:, 0:2, :], d_xT.ap()[0, :, 0:2, 0:TB])
            nc.sync.dma_start(wq_t[:, 2:4, :], d_wq.ap()[:, 2:4, :])
            nc.sync.dma_start(xg[0][:, 2:4, :], d_xT.ap()[0, :, 2:4, 0:TB])
            nc.sync.dma_start(xg[1][:], d_xT.ap()[0, :, 4:8, 0:TB])
            nc.sync.dma_start(wq_t[:, 4:, :], d_wq.ap()[:, 4:, :])
            nc.sync.dma_start(wk_t[:, 0:4, :], d_wk.ap()[:, 0:4, :])
            nc.sync.dma_start(xg[2][:], d_xT.ap()[0, :, 8:12, 0:TB])
            nc.sync.dma_start(wk_t[:, 4:, :], d_wk.ap()[:, 4:, :])
            nc.sync.dma_start(xg[3][:], d_xT.ap()[0, :, 12:16, 0:TB])
            wo_t = wp.tile([128, NH, C], bf16, name="wo_t")
            xts_pre = {}

            def _deferred_loads():
                # first needed well after the prologue; kept out of the early
                # DMA descriptor stream
                xts_pre[1] = emit_x(1)
                nc.sync.dma_start(wv_t[:], d_wv.ap()[:])
                for h in range(NH):
                    nc.sync.dma_start(
                        wo_t[:, h, :], d_wo.ap()[h * 128 : (h + 1) * 128, :]
                    )

            emit_qkv_proj(0, xts0, after_qk=_deferred_loads)

            att_order = list(range(8))
            att_queue = []
            proj_done = 0
            for step, ai in enumerate(att_order):
                # ensure projections for all blocks this attention needs
                while proj_done < ai:
                    nxt = proj_done + 1
                    if nxt not in xts_pre:
                        xts_pre[nxt] = emit_x(nxt)
                    emit_qkv_proj(nxt, xts_pre.pop(nxt))
                    proj_done = nxt
                if proj_done + 1 < len(blocks) and (proj_done + 1) not in xts_pre:
                    xts_pre[proj_done + 1] = emit_x(proj_done + 1)
                yTs = emit_attention(ai)
                emit_outproj(ai, yTs)
                if proj_done + 1 < len(blocks):
                    emit_qkv_proj(proj_done + 1, xts_pre.pop(proj_done + 1))
                    proj_done += 1

    nc.compile()
    return nc


def _get_program():
    if "nc" not in _STATE:
        _STATE["nc"] = _build_program()
    return _STATE["nc"]


def _enable_trace_hooks():
    import types

    import antenv

    if not hasattr(antenv, "axon_hooks"):
        hooks_mod = types.ModuleType("antenv.axon_hooks")
        _hook = [None]
        hooks_mod.set_axon_ntff_profile_hook = lambda h: _hook.__setitem__(0, h)
        hooks_mod.get_axon_ntff_profile_hook = lambda: _hook[0]
        sys.modules["antenv.axon_hooks"] = hooks_mod
        antenv.axon_hooks = hooks_mod
        from trn_agent_boot.trn_boot import _ntff_profile_via_ctypes

        hooks_mod.set_axon_ntff_profile_hook(
            _ntff_profile_via_ctypes("/opt/axon/libaxon_pjrt.so")
        )
    bass_utils.upload_artifacts = lambda tmpdir: f"local://{tmpdir}"


def kernel(x, Wqkv, bqkv, Wo, bo):
    global LAST_RESULT
    x = np.asarray(x, dtype=np.float32)
    Wqkv = np.asarray(Wqkv, dtype=np.float32)
    bqkv = np.asarray(bqkv, dtype=np.float32)
    Wo = np.asarray(Wo, dtype=np.float32)
    bo = np.asarray(bo, dtype=np.float32)

    nc = _get_program()

    cosT, sinT = _rope_tables()
    sinT = sinT.copy()
    sinT[: HD // 2, :] *= -1.0  # rotation sign folded into the sin table
    # rolled so both DVE operands share a partition base:
    # sin_swap[p] = sin[(p + 64) % 128]; t2[0:64] = qb[64:] * sin_swap[64:]
    sinT = np.roll(sinT, -64, axis=0)
    onesm = np.ones((128, 128), dtype=np.float32)
    # mask0[j, i] = 1 if key j <= query i within the diagonal 128x128 tile
    i_idx = np.arange(128)[None, :]
    j_idx = np.arange(128)[:, None]
    mask0 = (j_idx <= i_idx).astype(np.float32)
    xT = np.ascontiguousarray(
        x.transpose(0, 2, 1).reshape(B, CK, 128, T).transpose(0, 2, 1, 3)
    )

    in_maps = []
    for c in range(NC_):
        rs = slice(c * NH * HD, (c + 1) * NH * HD)
        in_maps.append(
            {
                "xT": xT.astype(BF16),
                "wq": np.ascontiguousarray(
                    Wqkv[0 * C :][rs.start : rs.stop, :].T.reshape(CK, 128, NH * HD).transpose(1, 0, 2)
                ).astype(BF16),
                "wk": np.ascontiguousarray(
                    Wqkv[1 * C :][rs.start : rs.stop, :].T.reshape(CK, 128, NH * HD).transpose(1, 0, 2)
                ).astype(BF16),
                "wv": np.ascontiguousarray(
                    Wqkv[2 * C :][rs.start : rs.stop, :].T.reshape(CK, 128, NH * HD).transpose(1, 0, 2)
                ).astype(BF16),
                "wo": np.ascontiguousarray(Wo[:, rs].T).astype(BF16),
                "bq": np.ascontiguousarray(bqkv[0 * C :][rs].reshape(NH, HD).T),
                "bk": np.ascontiguousarray(bqkv[1 * C :][rs].reshape(NH, HD).T),
                "cosT": cosT.astype(BF16),
                "sinT": sinT.astype(BF16),
                "mask0": mask0.astype(BF16),
                "onesm": onesm,
            }
        )

    if TRACE:
        _enable_trace_hooks()
    res = bass_utils.run_bass_kernel_spmd(
        nc, in_maps, core_ids=list(range(NC_)), trace=TRACE
    )
    LAST_RESULT = res

    out = np.zeros((B, T, C), dtype=np.float64)
    for c in range(NC_):
        out += res.results[c]["out"].astype(np.float64)
    bv = bqkv[2 * C : 3 * C]
    out += (bo + Wo @ bv)[None, None, :]
    return out.astype(np.float32)


# revision 21
# speedup vs baseline: 1.1894x; 1.1894x over previous
"""Causal self-attention (B=2, T=2048, C=2048, H=16) on 8 TRN2 NeuronCores.

Sharding: tensor-parallel over heads (2 heads per core, both batches on every
core). Each core computes q/k/v projections for its 2 heads, RoPE, causal
softmax(qk^T)v, and a partial output projection against its slice of Wo's
columns. The host sums the 8 partial projections and adds the (linear) bias
terms.

Layout strategy (evidence-driven: the PE streams at 1 cyc/row when fed; all
baseline loss was inter-instruction stalls — attention locally ACT-bound,
out-proj drain-bound, DMA descriptor generation serializing on the sync
engine, and slow casting DMAs starving prefetch):
  - everything on the PE path is bf16 (host-converted): x, Wq/Wk/Wv, q/k
    post-RoPE, exp(P), v, y, Wo. bf16 matmuls run 1 cyc/row at any N, halve
    LDWEIGHTS, and bf16 DVE ops run in 2x mode. PSUM accumulation is fp32.
  - emission order gives the Tile dataflow scheduler cross-phase overlap:
    attention(i) + out-proj(i) are emitted before qkv-proj(i+1), so the PE
    fills exp-latency gaps with next-block projection chains.
  - few, large DMA descriptors (the sync engine generates descriptors
    in-order at ~0.6us each): one descriptor per x block, two per weight
    tensor, block-0 x staged finer so the first matmul starts ASAP.
  - RoPE's half-rotation uses a host-rolled sin table and two
    partition-offset DVE multiplies — no SBUF-to-SBUF swap DMAs.
  - PSUM budget exactly 8 banks: ms=2 (scores/den/out-proj), my=2 (y acc),
    prj=2 (qk proj), vps=2 (v proj).
  - engine split: ACT = exp + q/k bias moves + half the PSUM drains,
    DVE = RoPE (bf16 2x) + dacc (f32r += bf16) + mask + recip + yT + rest.
  - denominator: dacc accumulated on DVE, then one ones-matmul broadcasts
    column sums to all partitions (exact fp32).
  - output written bf16 (halves store traffic); host sums 8 partials in f32;
    the last block stores per-512 chunk so the tail drain overlaps.
"""

import sys

sys.path.insert(0, "/opt/trn_rl_repo")

import numpy as np
import ml_dtypes

import concourse.bacc as bacc
import concourse.mybir as mybir
import concourse.tile as tile
from concourse import bass_utils

B, T, C, H = 2, 2048, 2048, 16
HD = C // H  # 128
BASE = 10000.0
NC_ = 8  # cores
NH = H // NC_  # heads per core = 2
TB = 512  # T block
NTB = T // TB  # 4
CK = C // 128  # 16 contraction chunks
SCALE = 1.0 / float(np.sqrt(np.float32(HD)))

f32 = mybir.dt.float32
f32r = mybir.dt.float32r
bf16 = mybir.dt.bfloat16
AF = mybir.ActivationFunctionType
OP = mybir.AluOpType
BF16 = ml_dtypes.bfloat16

TRACE = False
LAST_RESULT = None

_STATE = {}


def _rope_tables():
    """cos/sin tables [HD, T] mirroring reference._rope_tables (f32 chain)."""
    try:
        import jax
        import jax.numpy as jnp

        cpu = jax.devices("cpu")[0]
        with jax.default_device(cpu):
            p = jnp.arange(HD // 2, dtype=jnp.float32)
            theta = jnp.power(BASE, -(2.0**p) / HD)
            pos = jnp.arange(1, T + 1, dtype=jnp.float32)[:, None]
            c = pos * theta
            ang = jnp.concatenate([c, c], axis=-1)  # [T, HD]
            cos = np.asarray(jnp.cos(ang)).T  # [HD, T]
            sin = np.asarray(jnp.sin(ang)).T
        return np.ascontiguousarray(cos), np.ascontiguousarray(sin)
    except Exception:
        p = np.arange(HD // 2, dtype=np.float32)
        theta = np.power(np.float32(BASE), (-(2.0**p) / HD).astype(np.float32))
        pos = np.arange(1, T + 1, dtype=np.float32)[:, None]
        c = (pos * theta).astype(np.float32)
        ang = np.concatenate([c, c], axis=-1)
        return (
            np.ascontiguousarray(np.cos(ang).T.astype(np.float32)),
            np.ascontiguousarray(np.sin(ang).T.astype(np.float32)),
        )


def _build_program():
    nc = bacc.Bacc("TRN2", target_bir_lowering=False, debug=False, num_devices=NC_)

    d_xT = nc.dram_tensor("xT", (B, 128, CK, T), bf16, kind="ExternalInput")
    d_wq = nc.dram_tensor("wq", (128, CK, NH * HD), bf16, kind="ExternalInput")
    d_wk = nc.dram_tensor("wk", (128, CK, NH * HD), bf16, kind="ExternalInput")
    d_wv = nc.dram_tensor("wv", (128, CK, NH * HD), bf16, kind="ExternalInput")
    d_wo = nc.dram_tensor("wo", (NH * HD, C), bf16, kind="ExternalInput")
    d_bq = nc.dram_tensor("bq", (HD, NH), f32, kind="ExternalInput")
    d_bk = nc.dram_tensor("bk", (HD, NH), f32, kind="ExternalInput")
    d_cos = nc.dram_tensor("cosT", (HD, T), bf16, kind="ExternalInput")
    d_sin = nc.dram_tensor("sinT", (HD, T), bf16, kind="ExternalInput")
    d_mask = nc.dram_tensor("mask0", (128, 128), bf16, kind="ExternalInput")
    d_ones = nc.dram_tensor("onesm", (128, 128), f32r, kind="ExternalInput")
    d_out = nc.dram_tensor("out", (B, T, C), bf16, kind="ExternalOutput")

    blocks = [(b, tb) for b in range(B) for tb in range(NTB)]

    with tile.TileContext(nc) as tc:
        with (
            tc.tile_pool(name="w", bufs=1) as wp,
            tc.tile_pool(name="xp", bufs=1) as xp,
            tc.tile_pool(name="kv", bufs=1) as kvp,
            tc.tile_pool(name="work", bufs=1) as wk_,
            tc.tile_pool(name="ps", bufs=1, space="PSUM") as ps,
        ):
            # --- resident weights/constants (plain sync DMAs) ---
            wq_t = wp.tile([128, CK, NH * HD], bf16, name="wq_t")
            wk_t = wp.tile([128, CK, NH * HD], bf16, name="wk_t")
            wv_t = wp.tile([128, CK, NH * HD], bf16, name="wv_t")
            bq_t = wp.tile([128, NH], f32, name="bq_t")
            nc.sync.dma_start(bq_t[:], d_bq.ap()[:])
            bk_t = wp.tile([128, NH], f32, name="bk_t")
            nc.sync.dma_start(bk_t[:], d_bk.ap()[:])
            cos_t = wp.tile([128, T], bf16, name="cos_t")
            sin_t = wp.tile([128, T], bf16, name="sin_t")
            nc.sync.dma_start(cos_t[:], d_cos.ap()[:])
            nc.sync.dma_start(sin_t[:], d_sin.ap()[:])
            mask_t = wp.tile([128, 128], bf16, name="mask_t")
            nc.sync.dma_start(mask_t[:], d_mask.ap()[:])
            ones_t = wp.tile([128, 128], f32r, name="ones_t")
            nc.sync.dma_start(ones_t[:], d_ones.ap()[:])

            state = {}

            def emit_x(i):
                b, tb = blocks[i]
                tbs = slice(tb * TB, (tb + 1) * TB)
                xt = xp.tile([128, CK, TB], bf16, tag="xtb", bufs=2, name=f"xt_{i}")
                nc.sync.dma_start(xt[:], d_xT.ap()[b, :, :, tbs])
                return [xt[:, kc, :] for kc in range(CK)]

            def emit_qkv_proj(i, xts, after_qk=None):
                b, tb = blocks[i]
                tbs = slice(tb * TB, (tb + 1) * TB)
                if tb == 0:
                    state[("kts", b)] = [
                        kvp.tile(
                            [128, T], bf16, tag=f"kt{h}", bufs=2, name=f"kt{h}_{b}"
                        )
                        for h in range(NH)
                    ]
                    state[("vt", b)] = kvp.tile(
                        [128, T // 128, NH * HD], bf16, tag="v", bufs=2, name=f"v_{b}"
                    )
                kts = state[("kts", b)]
                vt = state[("vt", b)]
                qTs = []
                for h in range(NH):
                    hsl = slice(h * HD, (h + 1) * HD)
                    qT = wk_.tile([128, TB], bf16, tag="q", bufs=4)
                    for (w_t, b_t, dest) in (
                        (wq_t, bq_t, qT[:]),
                        (wk_t, bk_t, kts[h][:, tbs]),
                    ):
                        prj = ps.tile([128, TB], f32, tag="prj", bufs=2)
                        for kc in range(CK):
                            nc.tensor.matmul(
                                prj[:],
                                w_t[:, kc, hsl],
                                xts[kc],
                                start=(kc == 0),
                                stop=(kc == CK - 1),
                            )
                        qb = wk_.tile([128, TB], bf16, tag="qb", bufs=3)
                        nc.scalar.activation(
                            qb[:], prj[:], AF.Identity, bias=b_t[:, h : h + 1]
                        )
                        t1 = wk_.tile([128, TB], bf16, tag="rtmp", bufs=4)
                        nc.vector.tensor_tensor(t1[:], qb[:], cos_t[:, tbs], OP.mult)
                        t2 = wk_.tile([128, TB], bf16, tag="rtmp", bufs=4)
                        nc.vector.tensor_tensor(
                            t2[0:64, :], qb[64:128, :], sin_t[64:128, tbs], OP.mult
                        )
                        nc.vector.tensor_tensor(
                            t2[64:128, :], qb[0:64, :], sin_t[0:64, tbs], OP.mult
                        )
                        nc.vector.tensor_tensor(dest, t1[:], t2[:], OP.add)
                    qTs.append(qT)
                if after_qk is not None:
                    after_qk()
                # ---- v projection (both heads together, N=256) ----
                for tt in range(4):
                    vps = ps.tile([128, NH * HD], f32, tag="vps", bufs=2)
                    for kc in range(CK):
                        nc.tensor.matmul(
                            vps[:],
                            xts[kc][:, tt * 128 : (tt + 1) * 128],
                            wv_t[:, kc, :],
                            start=(kc == 0),
                            stop=(kc == CK - 1),
                        )
                    if tt % 2 == 0:
                        nc.scalar.activation(vt[:, tb * 4 + tt, :], vps[:], AF.Identity)
                    else:
                        nc.vector.tensor_copy(vt[:, tb * 4 + tt, :], vps[:])
                state[("qTs", i)] = qTs

            def emit_attention(i):
                b, tb = blocks[i]
                kts = state[("kts", b)]
                vt = state[("vt", b)]
                qTs = state.pop(("qTs", i))
                nkt = 4 * tb + 4
                # diagonal (W=512, masked) tile first so accumulation chains
                # start full-width; remaining diagonals at the end
                kt_order = (
                    [4 * tb] + list(range(4 * tb)) + [4 * tb + 1, 4 * tb + 2, 4 * tb + 3]
                )
                y_pss = [
                    ps.tile([128, TB], f32, tag="my", bufs=2, name=f"yps{h}_{i}")
                    for h in range(NH)
                ]
                daccs = [
                    wk_.tile([128, TB], f32r, tag="dacc", bufs=3, name=f"dacc{h}_{i}")
                    for h in range(NH)
                ]
                for idx, kt in enumerate(kt_order):
                    o = kt - 4 * tb
                    q0 = 128 * o if o > 0 else 0
                    W = TB - q0
                    first = idx == 0
                    last = idx == nkt - 1
                    for h in range(NH):
                        hsl = slice(h * HD, (h + 1) * HD)
                        s_ps = ps.tile([128, TB], f32, tag="ms", bufs=2)
                        nc.tensor.matmul(
                            s_ps[:, :W],
                            kts[h][:, kt * 128 : (kt + 1) * 128],
                            qTs[h][:, q0:],
                        )
                        pt = wk_.tile([128, TB], bf16, tag="p", bufs=6)
                        nc.scalar.activation(pt[:, :W], s_ps[:, :W], AF.Exp, scale=SCALE)
                        if o >= 0:
                            # triangular chunk is the first 128 live columns
                            nc.vector.tensor_tensor(
                                pt[:, :128], pt[:, :128], mask_t[:], OP.mult
                            )
                        nc.tensor.matmul(
                            y_pss[h][:, q0:],
                            vt[:, kt, hsl],
                            pt[:, :W],
                            start=first,
                            stop=last,
                            skip_group_check=True,
                        )
                        if first:
                            nc.vector.tensor_copy(daccs[h][:], pt[:])
                        else:
                            nc.vector.tensor_tensor(
                                daccs[h][:, q0:],
                                daccs[h][:, q0:].bitcast(f32),
                                pt[:, :W],
                                OP.add,
                            )
                yTs = []
                for h in range(NH):
                    den_ps = ps.tile([128, TB], f32, tag="ms", bufs=2)
                    nc.tensor.matmul(
                        den_ps[:], ones_t[:], daccs[h][:]
                    )
                    rden = wk_.tile([128, TB], f32, tag="rden", bufs=2)
                    nc.vector.reciprocal_approx_fast(rden[:], den_ps[:])
                    yT = wk_.tile([128, TB], bf16, tag="y", bufs=5)
                    nc.vector.tensor_tensor(yT[:], y_pss[h][:], rden[:], OP.mult)
                    yTs.append(yT)
                return yTs

            def emit_outproj(i, yTs):
                b, tb = blocks[i]
                last_block = i == len(blocks) - 1
                for tt in range(4):
                    r0 = tb * TB + tt * 128
                    ot = wk_.tile([128, C], bf16, tag="o", bufs=3)
                    for ncc in range(4):
                        o_ps = ps.tile([128, TB], f32, tag="ms", bufs=2)
                        for h in range(NH):
                            nc.tensor.matmul(
                                o_ps[:],
                                yTs[h][:, tt * 128 : (tt + 1) * 128],
                                wo_t[:, h, ncc * TB : (ncc + 1) * TB],
                                start=(h == 0),
                                stop=(h == NH - 1),
                            )
                        osl = ot[:, ncc * TB : (ncc + 1) * TB]
                        if ncc % 2 == 0:
                            nc.scalar.activation(osl, o_ps[:], AF.Identity)
                        else:
                            nc.vector.tensor_copy(osl, o_ps[:])
                        if last_block:
                            # drain per chunk so the final stores overlap the
                            # remaining copies instead of serializing after them
                            nc.sync.dma_start(
                                d_out.ap()[
                                    b, r0 : r0 + 128, ncc * TB : (ncc + 1) * TB
                                ],
                                osl,
                            )
                    if not last_block:
                        nc.sync.dma_start(d_out.ap()[b, r0 : r0 + 128, :], ot[:])

            # prologue: x(0) + wq/wk interleaved per chunk (fast time-to-first-
            # matmul), deferred wv/wo loads
            xts0 = []
            xg = []
            for kc0 in range(0, CK, 4):
                xt = xp.tile([128, 4, TB], bf16, tag="xt", bufs=8, name=f"xt0_{kc0}")
                xg.append(xt)
                xts0.extend(xt[:, g, :] for g in range(4))
            nc.sync.dma_start(wq_t[:, 0:2, :], d_wq.ap()[:, 0:2, :])
            nc.sync.dma_start(xg[0][:, 0:2, :], d_xT.ap()[0, :, 0:2, 0:TB])
            nc.sync.dma_start(wq_t[:, 2:4, :], d_wq.ap()[:, 2:4, :])
            nc.sync.dma_start(xg[0][:, 2:4, :], d_xT.ap()[0, :, 2:4, 0:TB])
            nc.sync.dma_start(xg[1][:], d_xT.ap()[0, :, 4:8, 0:TB])
            nc.sync.dma_start(wq_t[:, 4:, :], d_wq.ap()[:, 4:, :])
            nc.sync.dma_start(wk_t[:, 0:4, :], d_wk.ap()[:, 0:4, :])
            nc.sync.dma_start(xg[2][:], d_xT.ap()[0, :, 8:12, 0:TB])
            nc.sync.dma_start(wk_t[:, 4:, :], d_wk.ap()[:, 4:, :])
            nc.sync.dma_start(xg[3][:], d_xT.ap()[0, :, 12:16, 0:TB])
            wo_t = wp.tile([128, NH, C], bf16, name="wo_t")
            xts_pre = {}

            def _deferred_loads():
                # first needed well after the prologue; kept out of the early
                # DMA descriptor stream
                xts_pre[1] = emit_x(1)
                nc.sync.dma_start(wv_t[:], d_wv.ap()[:])
                for h in range(NH):
                    nc.sync.dma_start(
                        wo_t[:, h, :], d_wo.ap()[h * 128 : (h + 1) * 128, :]
                    )

            emit_qkv_proj(0, xts0, after_qk=_deferred_loads)

            att_order = list(range(8))
            att_queue = []
            proj_done = 0
            for step, ai in enumerate(att_order):
                # ensure projections for all blocks this attention needs
                while proj_done < ai:
                    nxt = proj_done + 1
                    if nxt not in xts_pre:
                        xts_pre[nxt] = emit_x(nxt)
                    emit_qkv_proj(nxt, xts_pre.pop(nxt))
                    proj_done = nxt
                if proj_done + 1 < len(blocks) and (proj_done + 1) not in xts_pre:
                    xts_pre[proj_done + 1] = emit_x(proj_done + 1)
                yTs = emit_attention(ai)
                emit_outproj(ai, yTs)
                if proj_done + 1 < len(blocks):
                    emit_qkv_proj(proj_done + 1, xts_pre.pop(proj_done + 1))
                    proj_done += 1

    nc.compile()
    return nc


def _get_program():
    if "nc" not in _STATE:
        _STATE["nc"] = _build_program()
    return _STATE["nc"]


def _enable_trace_hooks():
    import types

    import antenv

    if not hasattr(antenv, "axon_hooks"):
        hooks_mod = types.ModuleType("antenv.axon_hooks")
        _hook = [None]
        hooks_mod.set_axon_ntff_profile_hook = lambda h: _hook.__setitem__(0, h)
        hooks_mod.get_axon_ntff_profile_hook = lambda: _hook[0]
        sys.modules["antenv.axon_hooks"] = hooks_mod
        antenv.axon_hooks = hooks_mod
        from trn_agent_boot.trn_boot import _ntff_profile_via_ctypes

        hooks_mod.set_axon_ntff_profile_hook(
            _ntff_profile_via_ctypes("/opt/axon/libaxon_pjrt.so")
        )
    bass_utils.upload_artifacts = lambda tmpdir: f"local://{tmpdir}"


def kernel(x, Wqkv, bqkv, Wo, bo):
    global LAST_RESULT
    x = np.asarray(x, dtype=np.float32)
    Wqkv = np.asarray(Wqkv, dtype=np.float32)
    bqkv = np.asarray(bqkv, dtype=np.float32)
    Wo = np.asarray(Wo, dtype=np.float32)
    bo = np.asarray(bo, dtype=np.float32)

    nc = _get_program()

    cosT, sinT = _rope_tables()
    sinT = sinT.copy()
    sinT[: HD // 2, :] *= -1.0  # rotation sign folded into the sin table
    # rolled so both DVE operands share a partition base:
    # sin_swap[p] = sin[(p + 64) % 128]; t2[0:64] = qb[64:] * sin_swap[64:]
    sinT = np.roll(sinT, -64, axis=0)
    onesm = np.ones((128, 128), dtype=np.float32)
    # mask0[j, i] = 1 if key j <= query i within the diagonal 128x128 tile
    i_idx = np.arange(128)[None, :]
    j_idx = np.arange(128)[:, None]
    mask0 = (j_idx <= i_idx).astype(np.float32)
    xT = np.ascontiguousarray(
        x.transpose(0, 2, 1).reshape(B, CK, 128, T).transpose(0, 2, 1, 3)
    )

    in_maps = []
    for c in range(NC_):
        rs = slice(c * NH * HD, (c + 1) * NH * HD)
        in_maps.append(
            {
                "xT": xT.astype(BF16),
                "wq": np.ascontiguousarray(
                    Wqkv[0 * C :][rs.start : rs.stop, :].T.reshape(CK, 128, NH * HD).transpose(1, 0, 2)
                ).astype(BF16),
                "wk": np.ascontiguousarray(
                    Wqkv[1 * C :][rs.start : rs.stop, :].T.reshape(CK, 128, NH * HD).transpose(1, 0, 2)
                ).astype(BF16),
                "wv": np.ascontiguousarray(
                    Wqkv[2 * C :][rs.start : rs.stop, :].T.reshape(CK, 128, NH * HD).transpose(1, 0, 2)
                ).astype(BF16),
                "wo": np.ascontiguousarray(Wo[:, rs].T).astype(BF16),
                "bq": np.ascontiguousarray(bqkv[0 * C :][rs].reshape(NH, HD).T),
                "bk": np.ascontiguousarray(bqkv[1 * C :][rs].reshape(NH, HD).T),
                "cosT": cosT.astype(BF16),
                "sinT": sinT.astype(BF16),
                "mask0": mask0.astype(BF16),
                "onesm": onesm,
            }
        )

    if TRACE:
        _enable_trace_hooks()
    res = bass_utils.run_bass_kernel_spmd(
        nc, in_maps, core_ids=list(range(NC_)), trace=TRACE
    )
    LAST_RESULT = res

    out = np.zeros((B, T, C), dtype=np.float64)
    for c in range(NC_):
        out += res.results[c]["out"].astype(np.float64)
    bv = bqkv[2 * C : 3 * C]
    out += (bo + Wo @ bv)[None, None, :]
    return out.astype(np.float32)


# revision 22
# speedup vs baseline: 1.2169x; 1.0231x over previous
"""Causal self-attention (B=2, T=2048, C=2048, H=16) on 8 TRN2 NeuronCores.

Sharding: tensor-parallel over heads (2 heads per core, both batches on every
core). Each core computes q/k/v projections for its 2 heads, RoPE, causal
softmax(qk^T)v, and a partial output projection against its slice of Wo's
columns. The host sums the 8 partial projections and adds the (linear) bias
terms.

Layout strategy (evidence-driven: the PE streams at 1 cyc/row when fed; all
baseline loss was inter-instruction stalls — attention locally ACT-bound,
out-proj drain-bound, DMA descriptor generation serializing on the sync
engine, and slow casting DMAs starving prefetch):
  - everything on the PE path is bf16 (host-converted): x, Wq/Wk/Wv, q/k
    post-RoPE, exp(P), v, y, Wo. bf16 matmuls run 1 cyc/row at any N, halve
    LDWEIGHTS, and bf16 DVE ops run in 2x mode. PSUM accumulation is fp32.
  - emission order gives the Tile dataflow scheduler cross-phase overlap:
    attention(i) + out-proj(i) are emitted before qkv-proj(i+1), so the PE
    fills exp-latency gaps with next-block projection chains.
  - few, large DMA descriptors (the sync engine generates descriptors
    in-order at ~0.6us each): one descriptor per x block, two per weight
    tensor, block-0 x staged finer so the first matmul starts ASAP.
  - RoPE's half-rotation uses a host-rolled sin table and two
    partition-offset DVE multiplies — no SBUF-to-SBUF swap DMAs.
  - PSUM budget exactly 8 banks: ms=2 (scores/den/out-proj), my=2 (y acc),
    prj=2 (qk proj), vps=2 (v proj).
  - engine split: ACT = exp + q/k bias moves + half the PSUM drains,
    DVE = RoPE (bf16 2x) + dacc (f32r += bf16) + mask + recip + yT + rest.
  - denominator: dacc accumulated on DVE, then one ones-matmul broadcasts
    column sums to all partitions (exact fp32).
  - output written bf16 (halves store traffic); host sums 8 partials in f32;
    the last block stores per-512 chunk so the tail drain overlaps.
"""

import sys

sys.path.insert(0, "/opt/trn_rl_repo")

import numpy as np
import ml_dtypes

import concourse.bacc as bacc
import concourse.mybir as mybir
import concourse.tile as tile
from concourse import bass_utils

B, T, C, H = 2, 2048, 2048, 16
HD = C // H  # 128
BASE = 10000.0
NC_ = 8  # cores
NH = H // NC_  # heads per core = 2
TB = 512  # T block
NTB = T // TB  # 4
CK = C // 128  # 16 contraction chunks
SCALE = 1.0 / float(np.sqrt(np.float32(HD)))

f32 = mybir.dt.float32
f32r = mybir.dt.float32r
bf16 = mybir.dt.bfloat16
AF = mybir.ActivationFunctionType
OP = mybir.AluOpType
BF16 = ml_dtypes.bfloat16

TRACE = False
LAST_RESULT = None

_STATE = {}


def _rope_tables():
    """cos/sin tables [HD, T] mirroring reference._rope_tables (f32 chain)."""
    try:
        import jax
        import jax.numpy as jnp

        cpu = jax.devices("cpu")[0]
        with jax.default_device(cpu):
            p = jnp.arange(HD // 2, dtype=jnp.float32)
            theta = jnp.power(BASE, -(2.0**p) / HD)
            pos = jnp.arange(1, T + 1, dtype=jnp.float32)[:, None]
            c = pos * theta
            ang = jnp.concatenate([c, c], axis=-1)  # [T, HD]
            cos = np.asarray(jnp.cos(ang)).T  # [HD, T]
            sin = np.asarray(jnp.sin(ang)).T
        return np.ascontiguousarray(cos), np.ascontiguousarray(sin)
    except Exception:
        p = np.arange(HD // 2, dtype=np.float32)
        theta = np.power(np.float32(BASE), (-(2.0**p) / HD).astype(np.float32))
        pos = np.arange(1, T + 1, dtype=np.float32)[:, None]
        c = (pos * theta).astype(np.float32)
        ang = np.concatenate([c, c], axis=-1)
        return (
            np.ascontiguousarray(np.cos(ang).T.astype(np.float32)),
            np.ascontiguousarray(np.sin(ang).T.astype(np.float32)),
        )


def _build_program():
    nc = bacc.Bacc("TRN2", target_bir_lowering=False, debug=False, num_devices=NC_)

    d_xT = nc.dram_tensor("xT", (B, 128, CK, T), bf16, kind="ExternalInput")
    d_wq = nc.dram_tensor("wq", (128, CK, NH * HD), bf16, kind="ExternalInput")
    d_wk = nc.dram_tensor("wk", (128, CK, NH * HD), bf16, kind="ExternalInput")
    d_wv = nc.dram_tensor("wv", (128, CK, NH * HD), bf16, kind="ExternalInput")
    d_wo = nc.dram_tensor("wo", (NH * HD, C), bf16, kind="ExternalInput")
    d_bq = nc.dram_tensor("bq", (HD, NH), f32, kind="ExternalInput")
    d_bk = nc.dram_tensor("bk", (HD, NH), f32, kind="ExternalInput")
    d_cos = nc.dram_tensor("cosT", (HD, T), bf16, kind="ExternalInput")
    d_sin = nc.dram_tensor("sinT", (HD, T), bf16, kind="ExternalInput")
    d_mask = nc.dram_tensor("mask0", (128, 128), bf16, kind="ExternalInput")
    d_ones = nc.dram_tensor("onesm", (128, 128), f32r, kind="ExternalInput")
    d_out = nc.dram_tensor("out", (B, T, C), bf16, kind="ExternalOutput")

    blocks = [(b, tb) for b in range(B) for tb in range(NTB)]

    with tile.TileContext(nc) as tc:
        with (
            tc.tile_pool(name="w", bufs=1) as wp,
            tc.tile_pool(name="xp", bufs=1) as xp,
            tc.tile_pool(name="kv", bufs=1) as kvp,
            tc.tile_pool(name="work", bufs=1) as wk_,
            tc.tile_pool(name="ps", bufs=1, space="PSUM") as ps,
        ):
            # --- resident weights/constants (plain sync DMAs) ---
            wq_t = wp.tile([128, CK, NH * HD], bf16, name="wq_t")
            wk_t = wp.tile([128, CK, NH * HD], bf16, name="wk_t")
            wv_t = wp.tile([128, CK, NH * HD], bf16, name="wv_t")
            bq_t = wp.tile([128, NH], f32, name="bq_t")
            nc.sync.dma_start(bq_t[:], d_bq.ap()[:])
            bk_t = wp.tile([128, NH], f32, name="bk_t")
            nc.sync.dma_start(bk_t[:], d_bk.ap()[:])
            cos_t = wp.tile([128, T], bf16, name="cos_t")
            sin_t = wp.tile([128, T], bf16, name="sin_t")
            nc.sync.dma_start(cos_t[:], d_cos.ap()[:])
            nc.sync.dma_start(sin_t[:], d_sin.ap()[:])
            mask_t = wp.tile([128, 128], bf16, name="mask_t")
            nc.sync.dma_start(mask_t[:], d_mask.ap()[:])
            ones_t = wp.tile([128, 128], f32r, name="ones_t")
            nc.sync.dma_start(ones_t[:], d_ones.ap()[:])

            state = {}

            def emit_x(i):
                b, tb = blocks[i]
                tbs = slice(tb * TB, (tb + 1) * TB)
                xt = xp.tile([128, CK, TB], bf16, tag="xtb", bufs=2, name=f"xt_{i}")
                nc.sync.dma_start(xt[:], d_xT.ap()[b, :, :, tbs])
                return [xt[:, kc, :] for kc in range(CK)]

            def emit_qkv_proj(i, xts, after_qk=None):
                b, tb = blocks[i]
                tbs = slice(tb * TB, (tb + 1) * TB)
                if tb == 0:
                    state[("kts", b)] = [
                        kvp.tile(
                            [128, T], bf16, tag=f"kt{h}", bufs=2, name=f"kt{h}_{b}"
                        )
                        for h in range(NH)
                    ]
                    state[("vt", b)] = kvp.tile(
                        [128, T // 128, NH * HD], bf16, tag="v", bufs=2, name=f"v_{b}"
                    )
                kts = state[("kts", b)]
                vt = state[("vt", b)]
                qTs = []
                for h in range(NH):
                    hsl = slice(h * HD, (h + 1) * HD)
                    qT = wk_.tile([128, TB], bf16, tag="q", bufs=4)
                    for (w_t, b_t, dest) in (
                        (wq_t, bq_t, qT[:]),
                        (wk_t, bk_t, kts[h][:, tbs]),
                    ):
                        prj = ps.tile([128, TB], f32, tag="prj", bufs=2)
                        for kc in range(CK):
                            nc.tensor.matmul(
                                prj[:],
                                w_t[:, kc, hsl],
                                xts[kc],
                                start=(kc == 0),
                                stop=(kc == CK - 1),
                            )
                        qb = wk_.tile([128, TB], bf16, tag="qb", bufs=3)
                        nc.scalar.activation(
                            qb[:], prj[:], AF.Identity, bias=b_t[:, h : h + 1]
                        )
                        t1 = wk_.tile([128, TB], bf16, tag="rtmp", bufs=4)
                        nc.vector.tensor_tensor(t1[:], qb[:], cos_t[:, tbs], OP.mult)
                        t2 = wk_.tile([128, TB], bf16, tag="rtmp", bufs=4)
                        nc.vector.tensor_tensor(
                            t2[0:64, :], qb[64:128, :], sin_t[64:128, tbs], OP.mult
                        )
                        nc.vector.tensor_tensor(
                            t2[64:128, :], qb[0:64, :], sin_t[0:64, tbs], OP.mult
                        )
                        nc.vector.tensor_tensor(dest, t1[:], t2[:], OP.add)
                    qTs.append(qT)
                if after_qk is not None:
                    after_qk()
                # ---- v projection (both heads together, N=256) ----
                for tt in range(4):
                    vps = ps.tile([128, NH * HD], f32, tag="vps", bufs=1)
                    for kc in range(CK):
                        nc.tensor.matmul(
                            vps[:],
                            xts[kc][:, tt * 128 : (tt + 1) * 128],
                            wv_t[:, kc, :],
                            start=(kc == 0),
                            stop=(kc == CK - 1),
                        )
                    if tt % 2 == 0:
                        nc.scalar.activation(vt[:, tb * 4 + tt, :], vps[:], AF.Identity)
                    else:
                        nc.vector.tensor_copy(vt[:, tb * 4 + tt, :], vps[:])
                state[("qTs", i)] = qTs

            def emit_attention(i):
                b, tb = blocks[i]
                kts = state[("kts", b)]
                vt = state[("vt", b)]
                qTs = state.pop(("qTs", i))
                nkt = 4 * tb + 4
                # diagonal (W=512, masked) tile first so accumulation chains
                # start full-width; remaining diagonals at the end
                kt_order = (
                    [4 * tb] + list(range(4 * tb)) + [4 * tb + 1, 4 * tb + 2, 4 * tb + 3]
                )
                y_pss = [
                    ps.tile([128, TB], f32, tag="my", bufs=2, name=f"yps{h}_{i}")
                    for h in range(NH)
                ]
                daccs = [
                    wk_.tile([128, TB], f32r, tag="dacc", bufs=3, name=f"dacc{h}_{i}")
                    for h in range(NH)
                ]
                for idx, kt in enumerate(kt_order):
                    o = kt - 4 * tb
                    q0 = 128 * o if o > 0 else 0
                    W = TB - q0
                    first = idx == 0
                    last = idx == nkt - 1
                    for h in range(NH):
                        hsl = slice(h * HD, (h + 1) * HD)
                        s_ps = ps.tile([128, TB], f32, tag="ms", bufs=3)
                        nc.tensor.matmul(
                            s_ps[:, :W],
                            kts[h][:, kt * 128 : (kt + 1) * 128],
                            qTs[h][:, q0:],
                        )
                        pt = wk_.tile([128, TB], bf16, tag="p", bufs=6)
                        nc.scalar.activation(pt[:, :W], s_ps[:, :W], AF.Exp, scale=SCALE)
                        if o >= 0:
                            # triangular chunk is the first 128 live columns
                            nc.vector.tensor_tensor(
                                pt[:, :128], pt[:, :128], mask_t[:], OP.mult
                            )
                        nc.tensor.matmul(
                            y_pss[h][:, q0:],
                            vt[:, kt, hsl],
                            pt[:, :W],
                            start=first,
                            stop=last,
                            skip_group_check=True,
                        )
                        if first:
                            nc.vector.tensor_copy(daccs[h][:], pt[:])
                        else:
                            nc.vector.tensor_tensor(
                                daccs[h][:, q0:],
                                daccs[h][:, q0:].bitcast(f32),
                                pt[:, :W],
                                OP.add,
                            )
                yTs = []
                for h in range(NH):
                    den_ps = ps.tile([128, TB], f32, tag="ms", bufs=3)
                    nc.tensor.matmul(
                        den_ps[:], ones_t[:], daccs[h][:]
                    )
                    rden = wk_.tile([128, TB], f32, tag="rden", bufs=2)
                    nc.vector.reciprocal_approx_fast(rden[:], den_ps[:])
                    yT = wk_.tile([128, TB], bf16, tag="y", bufs=5)
                    nc.vector.tensor_tensor(yT[:], y_pss[h][:], rden[:], OP.mult)
                    yTs.append(yT)
                return yTs

            def emit_outproj(i, yTs):
                b, tb = blocks[i]
                last_block = i == len(blocks) - 1
                for tt in range(4):
                    r0 = tb * TB + tt * 128
                    ot = wk_.tile([128, C], bf16, tag="o", bufs=3)
                    for ncc in range(4):
                        o_ps = ps.tile([128, TB], f32, tag="ms", bufs=3)
                        for h in range(NH):
                            nc.tensor.matmul(
                                o_ps[:],
                                yTs[h][:, tt * 128 : (tt + 1) * 128],
                                wo_t[:, h, ncc * TB : (ncc + 1) * TB],
                                start=(h == 0),
                                stop=(h == NH - 1),
                            )
                        osl = ot[:, ncc * TB : (ncc + 1) * TB]
                        if ncc % 2 == 0:
                            nc.scalar.activation(osl, o_ps[:], AF.Identity)
                        else:
                            nc.vector.tensor_copy(osl, o_ps[:])
                        if last_block:
                            # drain per chunk so the final stores overlap the
                            # remaining copies instead of serializing after them
                            nc.sync.dma_start(
                                d_out.ap()[
                                    b, r0 : r0 + 128, ncc * TB : (ncc + 1) * TB
                                ],
                                osl,
                            )
                    if not last_block:
                        nc.sync.dma_start(d_out.ap()[b, r0 : r0 + 128, :], ot[:])

            # prologue: x(0) + wq/wk interleaved per chunk (fast time-to-first-
            # matmul), deferred wv/wo loads
            xts0 = []
            xg = []
            for kc0 in range(0, CK, 4):
                xt = xp.tile([128, 4, TB], bf16, tag="xt", bufs=8, name=f"xt0_{kc0}")
                xg.append(xt)
                xts0.extend(xt[:, g, :] for g in range(4))
            nc.sync.dma_start(wq_t[:, 0:2, :], d_wq.ap()[:, 0:2, :])
            nc.sync.dma_start(xg[0][:, 0:2, :], d_xT.ap()[0, :, 0:2, 0:TB])
            nc.sync.dma_start(wq_t[:, 2:4, :], d_wq.ap()[:, 2:4, :])
            nc.sync.dma_start(xg[0][:, 2:4, :], d_xT.ap()[0, :, 2:4, 0:TB])
            nc.sync.dma_start(xg[1][:], d_xT.ap()[0, :, 4:8, 0:TB])
            nc.sync.dma_start(wq_t[:, 4:, :], d_wq.ap()[:, 4:, :])
            nc.sync.dma_start(wk_t[:, 0:4, :], d_wk.ap()[:, 0:4, :])
            nc.sync.dma_start(xg[2][:], d_xT.ap()[0, :, 8:12, 0:TB])
            nc.sync.dma_start(wk_t[:, 4:, :], d_wk.ap()[:, 4:, :])
            nc.sync.dma_start(xg[3][:], d_xT.ap()[0, :, 12:16, 0:TB])
            wo_t = wp.tile([128, NH, C], bf16, name="wo_t")
            xts_pre = {}

            def _deferred_loads():
                # first needed well after the prologue; kept out of the early
                # DMA descriptor stream
                xts_pre[1] = emit_x(1)
                nc.sync.dma_start(wv_t[:], d_wv.ap()[:])
                for h in range(NH):
                    nc.sync.dma_start(
                        wo_t[:, h, :], d_wo.ap()[h * 128 : (h + 1) * 128, :]
                    )

            emit_qkv_proj(0, xts0, after_qk=_deferred_loads)

            att_order = list(range(8))
            att_queue = []
            proj_done = 0
            for step, ai in enumerate(att_order):
                # ensure projections for all blocks this attention needs
                while proj_done < ai:
                    nxt = proj_done + 1
                    if nxt not in xts_pre:
                        xts_pre[nxt] = emit_x(nxt)
                    emit_qkv_proj(nxt, xts_pre.pop(nxt))
                    proj_done = nxt
                if proj_done + 1 < len(blocks) and (proj_done + 1) not in xts_pre:
                    xts_pre[proj_done + 1] = emit_x(proj_done + 1)
                yTs = emit_attention(ai)
                emit_outproj(ai, yTs)
                if proj_done + 1 < len(blocks):
                    emit_qkv_proj(proj_done + 1, xts_pre.pop(proj_done + 1))
                    proj_done += 1

    nc.compile()
    return nc


def _get_program():
    if "nc" not in _STATE:
        _STATE["nc"] = _build_program()
    return _STATE["nc"]


def _enable_trace_hooks():
    import types

    import antenv

    if not hasattr(antenv, "axon_hooks"):
        hooks_mod = types.ModuleType("antenv.axon_hooks")
        _hook = [None]
        hooks_mod.set_axon_ntff_profile_hook = lambda h: _hook.__setitem__(0, h)
        hooks_mod.get_axon_ntff_profile_hook = lambda: _hook[0]
        sys.modules["antenv.axon_hooks"] = hooks_mod
        antenv.axon_hooks = hooks_mod
        from trn_agent_boot.trn_boot import _ntff_profile_via_ctypes

        hooks_mod.set_axon_ntff_profile_hook(
            _ntff_profile_via_ctypes("/opt/axon/libaxon_pjrt.so")
        )
    bass_utils.upload_artifacts = lambda tmpdir: f"local://{tmpdir}"


def kernel(x, Wqkv, bqkv, Wo, bo):
    global LAST_RESULT
    x = np.asarray(x, dtype=np.float32)
    Wqkv = np.asarray(Wqkv, dtype=np.float32)
    bqkv = np.asarray(bqkv, dtype=np.float32)
    Wo = np.asarray(Wo, dtype=np.float32)
    bo = np.asarray(bo, dtype=np.float32)

    nc = _get_program()

    cosT, sinT = _rope_tables()
    sinT = sinT.copy()
    sinT[: HD // 2, :] *= -1.0  # rotation sign folded into the sin table
    # rolled so both DVE operands share a partition base:
    # sin_swap[p] = sin[(p + 64) % 128]; t2[0:64] = qb[64:] * sin_swap[64:]
    sinT = np.roll(sinT, -64, axis=0)
    onesm = np.ones((128, 128), dtype=np.float32)
    # mask0[j, i] = 1 if key j <= query i within the diagonal 128x128 tile
    i_idx = np.arange(128)[None, :]
    j_idx = np.arange(128)[:, None]
    mask0 = (j_idx <= i_idx).astype(np.float32)
    xT = np.ascontiguousarray(
        x.transpose(0, 2, 1).reshape(B, CK, 128, T).transpose(0, 2, 1, 3)
    )

    in_maps = []
    for c in range(NC_):
        rs = slice(c * NH * HD, (c + 1) * NH * HD)
        in_maps.append(
            {
                "xT": xT.astype(BF16),
                "wq": np.ascontiguousarray(
                    Wqkv[0 * C :][rs.start : rs.stop, :].T.reshape(CK, 128, NH * HD).transpose(1, 0, 2)
                ).astype(BF16),
                "wk": np.ascontiguousarray(
                    Wqkv[1 * C :][rs.start : rs.stop, :].T.reshape(CK, 128, NH * HD).transpose(1, 0, 2)
                ).astype(BF16),
                "wv": np.ascontiguousarray(
                    Wqkv[2 * C :][rs.start : rs.stop, :].T.reshape(CK, 128, NH * HD).transpose(1, 0, 2)
                ).astype(BF16),
                "wo": np.ascontiguousarray(Wo[:, rs].T).astype(BF16),
                "bq": np.ascontiguousarray(bqkv[0 * C :][rs].reshape(NH, HD).T),
                "bk": np.ascontiguousarray(bqkv[1 * C :][rs].reshape(NH, HD).T),
                "cosT": cosT.astype(BF16),
                "sinT": sinT.astype(BF16),
                "mask0": mask0.astype(BF16),
                "onesm": onesm,
            }
        )

    if TRACE:
        _enable_trace_hooks()
    res = bass_utils.run_bass_kernel_spmd(
        nc, in_maps, core_ids=list(range(NC_)), trace=TRACE
    )
    LAST_RESULT = res

    out = np.zeros((B, T, C), dtype=np.float64)
    for c in range(NC_):
        out += res.results[c]["out"].astype(np.float64)
    bv = bqkv[2 * C : 3 * C]
    out += (bo + Wo @ bv)[None, None, :]
    return out.astype(np.float32)
